# revision 3
# baseline (speedup 1.0000x reference)
"""Trainium2 Bass kernel for nn_CARFACCell.

Math: per-row linear recurrence a[t] = f[t]*a[t-1] + g[t] over T=4096 (init
a0), followed by `steps` iterations of a symmetric-padded valid 5-tap
cross-correlation along T.  Rows = B*C = 4096, sharded 512 rows per core
across 8 cores (core b takes batch b); f/g/out travel over HBM as bf16.

Per-core schedule (HW-trace-driven):
  - f loads on the sync HWDGE ring, g loads on the scalar HWDGE ring, all
    issued up front with f/g fully resident in SBUF; the first tile is
    chunked (256,256,512,1024,2048) so the first scan chunk starts early.
  - DVE runs ONLY the chunked tensor_tensor_scan (~2.2-2.45 ns/col, the
    binding resource at ~37us/core); chunks chained through `initial`.
  - Conv: the composed `steps`-fold smoothing operator is a banded matrix
    (17-tap interior).  Per group of 8 windows: PE transposes into a bf16
    PSUM tile, one grouped ACT copy to SBUF, 8 matmuls against host-built
    W windows into a 2-bank fp32 PSUM tile at 128-col pitch (no matmul
    crosses a 2KB PSUM bank), one strided ACT cast to the bf16 out tile.
    Emission is software-pipelined (front of group k+1 before back of
    group k) so the PE FIFO never stalls on the ACT copies.
  - A dep-free warmup matmul run trips the PE HAM clock gate to 2.4 GHz
    before the first real transpose.
  - GpSimd does DMA only (consts + chunked SWDGE stores); its compute ops
    starve the DVE and are never used.
"""

import os
import numpy as np
import ml_dtypes

import concourse.bacc as bacc
import concourse.tile as tile
from concourse import mybir
from concourse.bass_utils import run_bass_kernel_spmd

B, C, T = 8, 512, 4096
N_CORES = 8
ROWS = B * C // N_CORES      # 512 rows per core
NRT = ROWS // 128            # 4 row-tiles per core
HALO = 8                     # steps * (K-1)//2 for steps=4, K=5
WIN = 128 - 2 * HALO         # 112 output columns per conv window
NW = (T + WIN - 1) // WIN    # 37 windows (last one overlaps)
F32 = mybir.dt.float32
BF16 = mybir.dt.bfloat16

_NC_CACHE = {}

DEF_CHUNKS = {
    0: (256, 256, 512, 1024, 2048),
    1: (2048, 2048),
    2: (2048, 2048),
    3: (1024, 1024, 1024, 1024),
}
DEF_PIECES = (2, 2, 2, 8)


def _build_w_windows(k5: np.ndarray, steps: int):
    """Build the three distinct [128, WIN] fp32 weight windows of the composed
    smoothing operator, numerically exact (including symmetric-pad boundary
    folds).  M[s, t] = d out[t] / d in[s]."""
    K = len(k5)
    pad = (K - 1) // 2
    assert steps * pad == HALO, (steps, K)
    L = 512
    M = np.eye(L, dtype=np.float64)
    k5 = np.asarray(k5, dtype=np.float64)
    for _ in range(steps):
        Mp = np.pad(M, ((0, 0), (pad, pad)), mode="symmetric")
        M = sum(k5[i] * Mp[:, i:i + L] for i in range(K))
    w_first = M[0:128, 0:WIN]
    w_mid = M[2 * WIN - HALO: 2 * WIN - HALO + 128, 2 * WIN: 3 * WIN]
    w_last = M[L - 128: L, L - WIN: L]
    return (np.ascontiguousarray(w_first, dtype=np.float32),
            np.ascontiguousarray(w_mid, dtype=np.float32),
            np.ascontiguousarray(w_last, dtype=np.float32))


def _window_geometry():
    """(t_in_start, t_out_start, out_col_start, psum_col_start, ncols, which_w)."""
    geo = []
    for j in range(NW):
        if j == 0:
            geo.append((0, 0, 0, 0, WIN, 0))
        elif j == NW - 1:
            to = T - WIN
            prev_end = WIN * (NW - 1)
            tail = T - prev_end                       # 64
            geo.append((T - 128, to, prev_end, WIN - tail, tail, 2))
        else:
            geo.append((WIN * j - HALO, WIN * j, WIN * j, 0, WIN, 1))
    return geo


def _build_nc(repeat=1, grp=8, chunks=None, store_pieces=DEF_PIECES,
              rec_bufs=3, out_bufs=2, cvp_bufs=2, xtp_bufs=3, xts_bufs=3,
              unroll=1, g_eng="scalar", store_eng="gpsimd",
              n_warm=40, g_sync_chunks=2):
    chunks = chunks or DEF_CHUNKS
    nc = bacc.Bacc("TRN2", target_bir_lowering=False, debug=False)

    f_d = nc.dram_tensor("f", [NRT, 128, T], BF16, kind="ExternalInput").ap()
    g_d = nc.dram_tensor("g", [NRT, 128, T], BF16, kind="ExternalInput").ap()
    a0_d = nc.dram_tensor("a0", [128, NRT], F32, kind="ExternalInput").ap()
    wf_d = nc.dram_tensor("w_first", [128, WIN], BF16, kind="ExternalInput").ap()
    wm_d = nc.dram_tensor("w_mid", [128, WIN], BF16, kind="ExternalInput").ap()
    wl_d = nc.dram_tensor("w_last", [128, WIN], BF16, kind="ExternalInput").ap()
    id_d = nc.dram_tensor("ident", [128, 128], BF16, kind="ExternalInput").ap()
    out_d = nc.dram_tensor("out", [NRT, 128, T], BF16, kind="ExternalOutput").ap()

    geo = _window_geometry()

    with tile.TileContext(nc) as tc:
        with (
            tc.tile_pool(name="const", bufs=1) as const_pool,
            tc.tile_pool(name="fg", bufs=NRT) as fg_pool,
            tc.tile_pool(name="rec", bufs=rec_bufs) as rec_pool,
            tc.tile_pool(name="outp", bufs=out_bufs) as out_pool,
            tc.tile_pool(name="xts", bufs=xts_bufs) as xts_pool,
            tc.tile_pool(name="cvp", bufs=cvp_bufs, space="PSUM") as cvp_pool,
            tc.tile_pool(name="xtp", bufs=xtp_bufs, space="PSUM") as xtp_pool,
        ):
            if n_warm:
                junk = const_pool.tile([128, WIN], BF16, tag="junk",
                                       name="junk")
                nc.gpsimd.memset(junk[:], 0)
                wrm = xtp_pool.tile([128, 512], F32, name="warm", tag="xtp")
                for _ in range(n_warm):
                    nc.tensor.matmul(wrm[0:WIN, 0:WIN], lhsT=junk[:],
                                     rhs=junk[:], start=True, stop=True)

            a0_t = const_pool.tile([128, NRT], F32, tag="a0", name="a0_t")
            nc.sync.dma_start(a0_t[:], a0_d)
            w_tiles = []
            for nm, d in (("wf", wf_d), ("wm", wm_d), ("wl", wl_d)):
                wt = const_pool.tile([128, WIN], BF16, tag=nm, name=nm)
                nc.gpsimd.dma_start(wt[:], d)
                w_tiles.append(wt)
            idt = const_pool.tile([128, 128], BF16, tag="idt", name="idt")
            nc.gpsimd.dma_start(idt[:], id_d)

            import contextlib
            rep_ctx = (tc.For_i(0, repeat, 1) if repeat > 1
                       else contextlib.nullcontext())
            with rep_ctx:
                for _u in range(unroll):
                    _body(nc, geo, w_tiles, idt, a0_t, f_d, g_d, out_d,
                          fg_pool, rec_pool, out_pool, xts_pool, cvp_pool,
                          xtp_pool, grp, chunks, store_pieces,
                          g_eng, store_eng, g_sync_chunks)

    nc.compile()
    return nc


def _body(nc, geo, w_tiles, idt, a0_t, f_d, g_d, out_d,
          fg_pool, rec_pool, out_pool, xts_pool, cvp_pool, xtp_pool,
          grp, chunks, store_pieces, g_eng, store_eng, g_sync_chunks):
    g_load = getattr(nc, g_eng)
    store = getattr(nc, store_eng)

    # ---- all loads up front: f on sync, g on g_eng ring (first
    # g_sync_chunks g-chunks ride sync so the first scan isn't gated on the
    # scalar ring's slower preamble) ----
    f_ts, g_ts = [], []
    for rt in range(NRT):
        f_t = fg_pool.tile([128, T], BF16, tag="f", name=f"f_t{rt}")
        g_t = fg_pool.tile([128, T], BF16, tag="g", name=f"g_t{rt}")
        f_ts.append(f_t)
        g_ts.append(g_t)
        lch = chunks[rt] if rt == 0 else [2048] * (T // 2048)
        c0 = 0
        for ci, ch in enumerate(lch):
            nc.sync.dma_start(f_t[:, c0:c0 + ch], f_d[rt, :, c0:c0 + ch])
            ge = nc.sync if (rt == 0 and ci < g_sync_chunks) else g_load
            ge.dma_start(g_t[:, c0:c0 + ch], g_d[rt, :, c0:c0 + ch])
            c0 += ch

    # ---- software-pipelined conv emission across all tiles ----
    state = {}
    carry = [None]

    def front(rt, w0):
        st = state[rt]
        wins = geo[w0:w0 + grp]
        nwin = len(wins)
        xtp = xtp_pool.tile([128, 128 * nwin], BF16, name="xtp", tag="xtp")
        for k, (ti, *_r) in enumerate(wins):
            nc.tensor.transpose(xtp[:, 128 * k:128 * (k + 1)],
                                st["rec"][:, ti:ti + 128], idt[:])
        xts = xts_pool.tile([128, 128 * nwin], BF16, name="xts", tag="xts")
        nc.scalar.copy(xts[:], xtp[:])
        return wins, nwin, xts

    def back(rt, wins, nwin, xts):
        st = state[rt]
        out_t = st["out"]
        cvp = cvp_pool.tile([128, 128 * nwin], F32, name="cvp", tag="cvp")
        for k, (ti, to, oc, pc, ncols, wsel) in enumerate(wins):
            nc.tensor.matmul(cvp[:, 128 * k:128 * k + WIN],
                             lhsT=xts[:, 128 * k:128 * (k + 1)],
                             rhs=w_tiles[wsel][:], start=True, stop=True)
        nreg = sum(1 for w in wins if not (w[5] == 2 and w[3] != 0))
        if nreg:
            oc0 = wins[0][2]
            src = cvp[:].rearrange("p (w c) -> p w c", w=nwin)[:, 0:nreg,
                                                              0:WIN]
            dst = out_t[:, oc0:oc0 + nreg * WIN].rearrange(
                "p (w c) -> p w c", w=nreg)
            nc.scalar.copy(dst, src)
        for k, w in enumerate(wins):
            (ti, to, oc, pc, ncols, wsel) = w
            if wsel == 2 and pc != 0:
                src = cvp[:, 128 * k + pc:128 * k + pc + ncols]
                nc.scalar.copy(out_t[:, oc:oc + ncols], src)
        done_hi = wins[-1][2] + wins[-1][4]
        if wins[-1][5] == 2:
            done_hi = T
        pc_len = st["pc_len"]
        while st["stored"] + pc_len <= done_hi:
            s0 = st["stored"]
            store.dma_start(out_d[rt, :, s0:s0 + pc_len],
                            out_t[:, s0:s0 + pc_len])
            st["stored"] += pc_len

    def emit_ready(rt, avail):
        st = state[rt]
        while st["done_w"] < NW:
            w0 = st["done_w"]
            wins = geo[w0:w0 + grp]
            hi = max(w[0] + 128 for w in wins)
            if hi > avail:
                break
            fr = front(rt, w0)
            if carry[0] is not None:
                carry[0]()
            carry[0] = (lambda rt=rt, fr=fr: back(rt, *fr))
            st["done_w"] += len(fr[0])

    for rt in range(NRT):
        f_t, g_t = f_ts[rt], g_ts[rt]
        rec_t = rec_pool.tile([128, T], BF16, name=f"rec{rt}", tag="rec")
        out_t = out_pool.tile([128, T], BF16, name=f"out{rt}", tag="out")
        npc = store_pieces[rt] if rt < len(store_pieces) else 2
        state[rt] = {"rec": rec_t, "out": out_t, "stored": 0,
                     "pc_len": T // npc, "done_w": 0}

        c0 = 0
        for ch in chunks[rt]:
            init = (a0_t[:, rt:rt + 1] if c0 == 0
                    else rec_t[:, c0 - 1:c0])
            nc.vector.tensor_tensor_scan(
                rec_t[:, c0:c0 + ch], f_t[:, c0:c0 + ch],
                g_t[:, c0:c0 + ch], initial=init,
                op0=mybir.AluOpType.mult, op1=mybir.AluOpType.add)
            c0 += ch
            emit_ready(rt, c0)
        if rt > 0:
            emit_ready(rt - 1, T)
    emit_ready(NRT - 1, T)
    if carry[0] is not None:
        carry[0]()
        carry[0] = None


def _fallback_numpy(a0, f, g, k5, steps):
    """Exact host-side computation for off-spec inputs (safety net)."""
    Bf, Cf, Tf = f.shape
    rows_f = f.reshape(-1, Tf).astype(np.float32)
    rows_g = g.reshape(-1, Tf).astype(np.float32)
    state = a0.reshape(-1).astype(np.float32).copy()
    rec = np.empty_like(rows_f)
    for t in range(Tf):
        state = rows_f[:, t] * state + rows_g[:, t]
        rec[:, t] = state
    k5 = np.asarray(k5, dtype=np.float32)
    Kk = len(k5)
    pad = (Kk - 1) // 2
    y = rec
    for _ in range(int(steps)):
        yp = np.pad(y, ((0, 0), (pad, pad)), mode="symmetric")
        y = sum(k5[i] * yp[:, i:i + Tf] for i in range(Kk))
    return y.reshape(Bf, Cf, Tf).astype(np.float32)


def _make_in_maps(a0, f, g, k5, steps):
    wf, wm, wl = _build_w_windows(np.asarray(k5, np.float64), int(steps))
    bf = ml_dtypes.bfloat16
    ident = np.eye(128, dtype=bf)
    in_maps = []
    for b in range(N_CORES):
        a0_dev = np.ascontiguousarray(
            a0[b].reshape(NRT, 128).T.astype(np.float32))
        in_maps.append({
            "f": np.ascontiguousarray(f[b].astype(bf)).reshape(NRT, 128, T),
            "g": np.ascontiguousarray(g[b].astype(bf)).reshape(NRT, 128, T),
            "a0": a0_dev,
            "w_first": wf.astype(bf),
            "w_mid": wm.astype(bf),
            "w_last": wl.astype(bf),
            "ident": ident,
        })
    return in_maps


def kernel(a0, f, g, kernel, steps):
    a0 = np.ascontiguousarray(np.asarray(a0), dtype=np.float32)
    f = np.ascontiguousarray(np.asarray(f), dtype=np.float32)
    g = np.ascontiguousarray(np.asarray(g), dtype=np.float32)
    k5 = np.asarray(kernel, dtype=np.float64)
    steps = int(steps)

    on_spec = (f.shape == (B, C, T) and g.shape == (B, C, T)
               and a0.shape == (B, C) and k5.shape == (5,)
               and steps * ((len(k5) - 1) // 2) == HALO)
    if not on_spec:
        return _fallback_numpy(a0, f, g, k5, steps)

    if "nc" not in _NC_CACHE:
        _NC_CACHE["nc"] = _build_nc()
    nc = _NC_CACHE["nc"]

    in_maps = _make_in_maps(a0, f, g, k5, steps)

    trace = os.environ.get("CARFAC_TRACE") == "1"
    if trace:
        try:
            import antenv.axon_hooks  # noqa: F401
        except ImportError:
            trace = False
    res = run_bass_kernel_spmd(nc, in_maps, list(range(N_CORES)), trace=trace)
    _NC_CACHE["last_res"] = res
    if trace and res.exec_time_ns is not None:
        print(f"HW exec time: {res.exec_time_ns} ns")
        _NC_CACHE["exec_time_ns"] = res.exec_time_ns

    out = np.stack(
        [np.asarray(res.results[b]["out"]).reshape(ROWS, T).astype(np.float32)
         for b in range(N_CORES)], axis=0)
    return out.reshape(B, C, T)


def bench_repeat(a0, f, g, kernel, steps, repeats=(1, 257), iters=6,
                 variant=None):
    """Estimate per-iteration HW time via a hardware For_i repeat loop and
    dispatch-wall differencing (axon overhead cancels)."""
    variant = dict(variant or {})
    variant.setdefault("unroll", 4)
    results = {}
    for rep in repeats:
        _NC_CACHE.pop("nc", None)
        _NC_CACHE["nc"] = _build_nc(repeat=rep, **variant)
        tmin, times, _ = bench(a0, f, g, kernel, steps, iters=iters)
        results[rep] = tmin
        print(f"repeat={rep}: min wall {tmin*1e3:.2f} ms "
              f"(all: {', '.join(f'{t*1e3:.1f}' for t in times)})")
    _NC_CACHE.pop("nc", None)
    reps = sorted(results)
    if len(reps) >= 2:
        r0, r1 = reps[0], reps[-1]
        per = (results[r1] - results[r0]) / (r1 - r0)
        per /= variant.get("unroll", 1)
        print(f"per-iteration HW time: {per*1e9:.0f} ns")
        return per
    return None


def bench(a0, f, g, kernel, steps, iters=10):
    """Time the sharded PJRT executable with device-resident inputs.
    Returns (min_wall_s, all_times, out). Not used by grading."""
    import time
    import jax
    from jax.sharding import Mesh, PartitionSpec
    from jax.experimental.shard_map import shard_map
    from concourse import bass2jax, mybir as _mybir

    a0 = np.asarray(a0, np.float32)
    f = np.asarray(f, np.float32)
    g = np.asarray(g, np.float32)

    if "nc" not in _NC_CACHE:
        _NC_CACHE["nc"] = _build_nc()
    nc = _NC_CACHE["nc"]

    bass2jax.install_neuronx_cc_hook()
    partition_name = (nc.partition_id_tensor.name
                      if nc.partition_id_tensor else None)
    in_names, out_names, out_avals, zero_outs = [], [], [], []
    for alloc in nc.m.functions[0].allocations:
        if not isinstance(alloc, _mybir.MemoryLocationSet):
            continue
        name = alloc.memorylocations[0].name
        if alloc.kind == "ExternalInput":
            if name != partition_name:
                in_names.append(name)
        elif alloc.kind == "ExternalOutput":
            shape = tuple(alloc.tensor_shape)
            dtype = _mybir.dt.np(alloc.dtype)
            out_names.append(name)
            out_avals.append(jax.core.ShapedArray(shape, dtype))
            zero_outs.append(np.zeros(shape, dtype))
    n_params = len(in_names)
    n_outs = len(out_avals)
    all_in_names = list(in_names) + out_names
    if partition_name is not None:
        all_in_names.append(partition_name)

    def _jbody(*args):
        operands = list(args)
        if partition_name is not None:
            operands.append(bass2jax.partition_id_tensor())
        outs = bass2jax._bass_exec_p.bind(
            *operands,
            out_avals=tuple(out_avals),
            in_names=tuple(all_in_names),
            out_names=tuple(out_names),
            lowering_input_output_aliases=(),
            sim_require_finite=True,
            sim_require_nnan=True,
            nc=nc,
        )
        return tuple(outs)

    in_maps = _make_in_maps(a0, f, g, kernel, steps)
    per_core = [[np.asarray(m[nm]) for nm in in_names] for m in in_maps]
    concat_in = [np.concatenate([per_core[c][i] for c in range(N_CORES)],
                                axis=0) for i in range(n_params)]
    concat_zeros = [np.zeros((N_CORES * z.shape[0], *z.shape[1:]), z.dtype)
                    for z in zero_outs]

    devices = jax.devices()[:N_CORES]
    mesh = Mesh(np.asarray(devices), ("core",))
    sharded = jax.jit(
        shard_map(_jbody, mesh=mesh,
                  in_specs=(PartitionSpec("core"),) * (n_params + n_outs),
                  out_specs=(PartitionSpec("core"),) * n_outs,
                  check_rep=False),
        donate_argnums=tuple(range(n_params, n_params + n_outs)),
        keep_unused=True,
    )

    in_dev = [jax.device_put(x) for x in concat_in]
    times = []
    out = None
    for _ in range(iters):
        zeros_dev = [jax.device_put(z) for z in concat_zeros]
        jax.block_until_ready(zeros_dev)
        jax.block_until_ready(in_dev)
        t0 = time.perf_counter()
        out = sharded(*in_dev, *zeros_dev)
        jax.block_until_ready(out)
        times.append(time.perf_counter() - t0)
    return min(times), times, out
